# revision 5
# baseline (speedup 1.0000x reference)
"""Trainium2 Bass kernel for nn_MetricPoseLoss: Gumbel top-k match sampling +
RANSAC/Procrustes hypothesis scoring, data-parallel over 8 NeuronCores.

Host side: replicates the reference's Gumbel noise (jax threefry, CPU backend)
and logm = log(matches+1e-12); streams v = logm + gumbel to the device.
Device side (per core, 4 batch elems x 4 sampling iterations = 16 rows):
stream v row tiles, per-partition gumbel-top-4 selection (512 samples/row) via
vector max8/max_index, indirect-DMA gathers of backprojected keypoint pairs
and log-weights, then 8 RANSAC hypotheses per row: gumbel-top-5 minimal sets,
Horn-quaternion weighted Procrustes, inlier scoring, pose loss, and
softmax-with-null combine. Output [32,1] f32.
"""
import os
import numpy as np

B, NK = 32, 1024
S = 512
ITM, ITR = 4, 8
C5 = 5
TH3D = 0.15
BETA = 5.0 / TH3D
TEMP = 10.0
THOUT = 0.35
MAXNULL = 0.5
SCM = 0.5
P = 128
FREE = NK * NK // P  # 8192
NCORES = 8
BPC = B // NCORES    # 4 batches per core
ROWS = BPC * ITM     # 16 rows per core
NULLSCORE = float(np.float32(THOUT) * np.float32(S))

_NC_CACHE = {}


def _build_nc():
    if "nc" in _NC_CACHE:
        return _NC_CACHE["nc"]
    import concourse.bacc as bacc
    import concourse.mybir as mybir
    import concourse.tile as tile
    from concourse.bass import IndirectOffsetOnAxis

    dt = mybir.dt
    op = mybir.AluOpType
    AF = mybir.ActivationFunctionType

    nc = bacc.Bacc("TRN2", target_bir_lowering=False, debug=False,
                   num_devices=NCORES)
    vrows_d = nc.dram_tensor("vrows", [ROWS, P, FREE], dt.float32, kind="ExternalInput")
    logm_d = nc.dram_tensor("logm4", [BPC * NK * NK, 1], dt.float32, kind="ExternalInput")
    tab0_d = nc.dram_tensor("tab0", [BPC * NK, 4], dt.float32, kind="ExternalInput")
    tab1_d = nc.dram_tensor("tab1", [BPC * NK, 4], dt.float32, kind="ExternalInput")
    gk_d = nc.dram_tensor("gk", [P, S], dt.float32, kind="ExternalInput")
    rgt_d = nc.dram_tensor("rgt", [P, 12], dt.float32, kind="ExternalInput")
    out_d = nc.dram_tensor("out", [BPC, 1], dt.float32, kind="ExternalOutput")
    xrow_d = nc.dram_tensor("xrow", [ROWS, S, 4], dt.float32, kind="Internal")
    yrow_d = nc.dram_tensor("yrow", [ROWS, S, 4], dt.float32, kind="Internal")
    lrow_d = nc.dram_tensor("lrow", [ROWS, S], dt.float32, kind="Internal")
    t16_d = nc.dram_tensor("t16", [ROWS, 1], dt.float32, kind="Internal")

    with tile.TileContext(nc) as tc:
        with (
            tc.tile_pool(name="vpool", bufs=2) as vpool,
            tc.tile_pool(name="sel", bufs=2) as sel,
            tc.tile_pool(name="cst", bufs=1) as cst,
            tc.tile_pool(name="hyp", bufs=1) as hyp,
            tc.tile_pool(name="tmp", bufs=2) as tmp,
            tc.tile_pool(name="ps", bufs=2, space="PSUM") as ps,
        ):
            # constants
            pbase = cst.tile([P, 1], dt.int32)
            nc.gpsimd.iota(pbase[:], [[0, 1]], base=0, channel_multiplier=FREE)
            pbasef = cst.tile([P, 1], dt.float32)
            nc.vector.tensor_copy(pbasef[:], pbase[:])
            ones1 = cst.tile([P, 1], dt.float32)
            nc.vector.memset(ones1[:], 1.0)
            b5 = cst.tile([P, 1], dt.float32)
            nc.vector.memset(b5[:], float(np.float32(BETA) * np.float32(TH3D)))
            b0 = cst.tile([P, 1], dt.float32)
            nc.vector.memset(b0[:], 0.0)
            b0s = cst.tile([16, 1], dt.float32)
            nc.vector.memset(b0s[:], 0.0)

            # ---------- per-row selection + gathers ----------
            for r in range(ROWS):
                bc = r // ITM
                vt = vpool.tile([P, FREE], dt.float32, tag="vt")
                nc.sync.dma_start(vt[:], vrows_d[r])
                m8 = sel.tile([P, 8], dt.float32, tag="m8")
                nc.vector.max(m8[:], vt[:])
                j8 = sel.tile([P, 8], dt.uint32, tag="j8")
                nc.vector.max_index(j8[:], m8[:], vt[:])
                jf = sel.tile([P, 4], dt.float32, tag="jf")
                nc.vector.tensor_copy(jf[:], j8[:, 0:4])
                gidxf = sel.tile([P, 4], dt.float32, tag="gidxf")
                nc.vector.tensor_scalar(out=gidxf[:], in0=jf[:], scalar1=pbasef[:, 0:1],
                                        scalar2=None, op0=op.add)
                gidxi = sel.tile([P, 4], dt.int32, tag="gidxi")
                nc.vector.tensor_copy(gidxi[:], gidxf[:])
                # i0 = floor(gidx/1024) via round-nearest cast of x/1024 - 0.49951171875
                t1 = sel.tile([P, 4], dt.float32, tag="t1")
                nc.vector.tensor_scalar(out=t1[:], in0=gidxf[:], scalar1=float(1.0 / 1024.0),
                                        scalar2=-0.49951171875, op0=op.mult, op1=op.add)
                i0i = sel.tile([P, 4], dt.int32, tag="i0i")
                nc.vector.tensor_copy(i0i[:], t1[:])
                i0f = sel.tile([P, 4], dt.float32, tag="i0f")
                nc.vector.tensor_copy(i0f[:], i0i[:])
                i1f = sel.tile([P, 4], dt.float32, tag="i1f")
                nc.vector.scalar_tensor_tensor(out=i1f[:], in0=i0f[:], scalar=-1024.0,
                                               in1=gidxf[:], op0=op.mult, op1=op.add)
                i1i = sel.tile([P, 4], dt.int32, tag="i1i")
                nc.vector.tensor_copy(i1i[:], i1f[:])

                lw4 = sel.tile([P, 4, 1], dt.float32, tag="lw4")
                xg = sel.tile([P, 4, 4], dt.float32, tag="xg")
                yg = sel.tile([P, 4, 4], dt.float32, tag="yg")
                for s in range(4):
                    nc.gpsimd.indirect_dma_start(
                        out=lw4[:, s, :], out_offset=None,
                        in_=logm_d[:],
                        in_offset=IndirectOffsetOnAxis(ap=gidxi[:, s:s + 1], axis=0),
                        element_offset=bc * NK * NK,
                        bounds_check=NK * NK - 1, oob_is_err=False)
                    nc.gpsimd.indirect_dma_start(
                        out=xg[:, s, :], out_offset=None,
                        in_=tab0_d[:],
                        in_offset=IndirectOffsetOnAxis(ap=i0i[:, s:s + 1], axis=0),
                        element_offset=bc * NK * 4,
                        bounds_check=NK - 1, oob_is_err=False)
                    nc.gpsimd.indirect_dma_start(
                        out=yg[:, s, :], out_offset=None,
                        in_=tab1_d[:],
                        in_offset=IndirectOffsetOnAxis(ap=i1i[:, s:s + 1], axis=0),
                        element_offset=bc * NK * 4,
                        bounds_check=NK - 1, oob_is_err=False)
                nc.sync.dma_start(xrow_d[r], xg[:])
                nc.sync.dma_start(yrow_d[r], yg[:])
                nc.sync.dma_start(lrow_d[r], lw4[:, :, 0])

            # ---------- hypothesis phase ----------
            xh = hyp.tile([P, S, 4], dt.float32)
            yh = hyp.tile([P, S, 4], dt.float32)
            lwh = hyp.tile([P, S], dt.float32)
            for r in range(ROWS):
                for k in range(8):
                    nc.sync.dma_start(xh[8 * r + k:8 * r + k + 1, :, :], xrow_d[r])
                    nc.sync.dma_start(yh[8 * r + k:8 * r + k + 1, :, :], yrow_d[r])
                    nc.sync.dma_start(lwh[8 * r + k:8 * r + k + 1, :], lrow_d[r])
            gk = hyp.tile([P, S], dt.float32)
            nc.sync.dma_start(gk[:], gk_d[:])
            rgt = hyp.tile([P, 12], dt.float32)
            nc.sync.dma_start(rgt[:], rgt_d[:])

            v5 = tmp.tile([P, S], dt.float32)
            nc.vector.tensor_tensor(out=v5[:], in0=lwh[:], in1=gk[:], op=op.add)
            m8b = tmp.tile([P, 8], dt.float32)
            nc.vector.max(m8b[:], v5[:])
            mask = tmp.tile([P, S], dt.float32)
            nc.vector.tensor_scalar(out=mask[:], in0=v5[:], scalar1=m8b[:, 4:5],
                                    scalar2=None, op0=op.is_ge)

            junk = tmp.tile([P, S], dt.float32)
            X = [xh[:, :, i] for i in range(3)]
            Y = [yh[:, :, i] for i in range(3)]

            def wproc(w):
                """weighted procrustes with weights w [P,S]; returns (R9, t3)."""
                wsum = tmp.tile([P, 1], dt.float32, tag="wsum")
                nc.vector.tensor_scalar(out=junk[:], in0=w[:], scalar1=1.0,
                                        scalar2=0.0, op0=op.mult, op1=op.add,
                                        accum_out=wsum[:])
                winv = tmp.tile([P, 1], dt.float32, tag="winv")
                nc.vector.reciprocal(winv[:], wsum[:])
                mu = tmp.tile([P, 6], dt.float32, tag="mu")
                for i in range(3):
                    nc.vector.scalar_tensor_tensor(out=junk[:], in0=X[i], scalar=1.0,
                                                   in1=w[:], op0=op.mult, op1=op.mult,
                                                   accum_out=mu[:, i:i + 1])
                    nc.vector.scalar_tensor_tensor(out=junk[:], in0=Y[i], scalar=1.0,
                                                   in1=w[:], op0=op.mult, op1=op.mult,
                                                   accum_out=mu[:, 3 + i:4 + i])
                nc.vector.tensor_scalar(out=mu[:], in0=mu[:], scalar1=winv[:, 0:1],
                                        scalar2=None, op0=op.mult)
                xc = tmp.tile([P, 3, S], dt.float32, tag="xc")
                yc = tmp.tile([P, 3, S], dt.float32, tag="yc")
                for i in range(3):
                    nc.vector.tensor_scalar(out=xc[:, i, :], in0=X[i], scalar1=mu[:, i:i + 1],
                                            scalar2=None, op0=op.subtract)
                    nc.vector.tensor_scalar(out=yc[:, i, :], in0=Y[i], scalar1=mu[:, 3 + i:4 + i],
                                            scalar2=None, op0=op.subtract)
                    nc.vector.tensor_tensor(out=xc[:, i, :], in0=xc[:, i, :], in1=w[:], op=op.mult)
                H = tmp.tile([P, 9], dt.float32, tag="H")
                for i in range(3):
                    for j in range(3):
                        nc.vector.scalar_tensor_tensor(
                            out=junk[:], in0=xc[:, i, :], scalar=1.0, in1=yc[:, j, :],
                            op0=op.mult, op1=op.mult, accum_out=H[:, 3 * i + j:3 * i + j + 1])
                nc.vector.tensor_scalar(out=H[:], in0=H[:], scalar1=winv[:, 0:1],
                                        scalar2=None, op0=op.mult)
                # Horn N matrix [P,16]
                N = tmp.tile([P, 16], dt.float32, tag="N")
                h = lambda i, j: H[:, 3 * i + j:3 * i + j + 1]
                def setn(k, expr_build):
                    expr_build(N[:, k:k + 1])
                def add2(dst, a, b, sa=1.0, sb=1.0):
                    nc.vector.scalar_tensor_tensor(out=dst, in0=a, scalar=sa, in1=junk[:, 0:1],
                                                   op0=op.mult, op1=op.bypass) if False else None
                # simple helpers with TT ops
                def lin(dst, a, b, sb):
                    # dst = a + sb*b
                    nc.vector.scalar_tensor_tensor(out=dst, in0=b, scalar=sb, in1=a,
                                                   op0=op.mult, op1=op.add)
                tr2 = tmp.tile([P, 4], dt.float32, tag="tr2")
                lin(tr2[:, 0:1], h(0, 0), h(1, 1), 1.0)
                lin(N[:, 0:1], tr2[:, 0:1], h(2, 2), 1.0)        # S00+S11+S22
                lin(N[:, 1:2], h(1, 2), h(2, 1), -1.0)           # S12-S21
                lin(N[:, 2:3], h(2, 0), h(0, 2), -1.0)           # S20-S02
                lin(N[:, 3:4], h(0, 1), h(1, 0), -1.0)           # S01-S10
                nc.vector.tensor_copy(N[:, 4:5], N[:, 1:2])
                lin(tr2[:, 1:2], h(0, 0), h(1, 1), -1.0)
                lin(N[:, 5:6], tr2[:, 1:2], h(2, 2), -1.0)       # S00-S11-S22
                lin(N[:, 6:7], h(0, 1), h(1, 0), 1.0)            # S01+S10
                lin(N[:, 7:8], h(0, 2), h(2, 0), 1.0)            # S02+S20
                nc.vector.tensor_copy(N[:, 8:9], N[:, 2:3])
                nc.vector.tensor_copy(N[:, 9:10], N[:, 6:7])
                lin(tr2[:, 2:3], h(1, 1), h(0, 0), -1.0)
                lin(N[:, 10:11], tr2[:, 2:3], h(2, 2), -1.0)     # -S00+S11-S22
                lin(N[:, 11:12], h(1, 2), h(2, 1), 1.0)          # S12+S21
                nc.vector.tensor_copy(N[:, 12:13], N[:, 3:4])
                nc.vector.tensor_copy(N[:, 13:14], N[:, 7:8])
                nc.vector.tensor_copy(N[:, 14:15], N[:, 11:12])
                lin(tr2[:, 3:4], h(2, 2), h(0, 0), -1.0)
                lin(N[:, 15:16], tr2[:, 3:4], h(1, 1), -1.0)     # -S00-S11+S22
                # shift: sigma = 2*sum|H|
                habs = tmp.tile([P, 9], dt.float32, tag="habs")
                nc.scalar.activation(habs[:], H[:], AF.Abs, bias=b0[:, 0:1], scale=1.0)
                sig = tmp.tile([P, 1], dt.float32, tag="sig")
                nc.vector.tensor_scalar(out=habs[:], in0=habs[:], scalar1=2.0,
                                        scalar2=0.0, op0=op.mult, op1=op.add,
                                        accum_out=sig[:])
                for k in (0, 5, 10, 15):
                    nc.vector.tensor_tensor(out=N[:, k:k + 1], in0=N[:, k:k + 1],
                                            in1=sig[:], op=op.add)
                q = tmp.tile([P, 4], dt.float32, tag="q")
                nc.vector.memset(q[:], 0.5)
                qn = tmp.tile([P, 4], dt.float32, tag="qn")
                ss = tmp.tile([P, 1], dt.float32, tag="ss")
                for _ in range(20):
                    for i in range(4):
                        nc.vector.tensor_scalar(out=qn[:, i:i + 1], in0=N[:, 4 * i:4 * i + 1],
                                                scalar1=q[:, 0:1], scalar2=None, op0=op.mult)
                        for j in range(1, 4):
                            nc.vector.scalar_tensor_tensor(
                                out=qn[:, i:i + 1], in0=N[:, 4 * i + j:4 * i + j + 1],
                                scalar=q[:, j:j + 1], in1=qn[:, i:i + 1],
                                op0=op.mult, op1=op.add)
                    nc.vector.scalar_tensor_tensor(out=junk[:, 0:4], in0=qn[:], scalar=1.0,
                                                   in1=qn[:], op0=op.mult, op1=op.mult,
                                                   accum_out=ss[:])
                    nc.vector.reciprocal(ss[:], ss[:])
                    nc.scalar.activation(ss[:], ss[:], AF.Sqrt, bias=b0[:, 0:1], scale=1.0)
                    nc.vector.tensor_scalar(out=q[:], in0=qn[:], scalar1=ss[:, 0:1],
                                            scalar2=None, op0=op.mult)
                # R from q
                pr = tmp.tile([P, 10], dt.float32, tag="pr")
                pairs = [(0, 0), (1, 1), (2, 2), (3, 3), (1, 2), (1, 3), (2, 3),
                         (0, 1), (0, 2), (0, 3)]
                for k, (a, bq) in enumerate(pairs):
                    nc.vector.tensor_scalar(out=pr[:, k:k + 1], in0=q[:, a:a + 1],
                                            scalar1=q[:, bq:bq + 1], scalar2=None, op0=op.mult)
                R9 = tmp.tile([P, 9], dt.float32, tag="R9")
                ww, xx, yy, zz = 0, 1, 2, 3
                xy, xz, yz = 4, 5, 6
                wx, wy, wz = 7, 8, 9
                def rset(k, p1, p2, s2, diag=False):
                    if diag:
                        # 1 - 2*(p1+p2)
                        nc.vector.tensor_tensor(out=R9[:, k:k + 1], in0=pr[:, p1:p1 + 1],
                                                in1=pr[:, p2:p2 + 1], op=op.add)
                        nc.vector.tensor_scalar(out=R9[:, k:k + 1], in0=R9[:, k:k + 1],
                                                scalar1=-2.0, scalar2=1.0,
                                                op0=op.mult, op1=op.add)
                    else:
                        # 2*(p1 + s2*p2)
                        nc.vector.scalar_tensor_tensor(out=R9[:, k:k + 1],
                                                       in0=pr[:, p2:p2 + 1], scalar=s2,
                                                       in1=pr[:, p1:p1 + 1],
                                                       op0=op.mult, op1=op.add)
                        nc.vector.tensor_scalar(out=R9[:, k:k + 1], in0=R9[:, k:k + 1],
                                                scalar1=2.0, scalar2=None, op0=op.mult)
                rset(0, yy, zz, 0, diag=True)
                rset(1, xy, wz, -1.0)
                rset(2, xz, wy, 1.0)
                rset(3, xy, wz, 1.0)
                rset(4, xx, zz, 0, diag=True)
                rset(5, yz, wx, -1.0)
                rset(6, xz, wy, -1.0)
                rset(7, yz, wx, 1.0)
                rset(8, xx, yy, 0, diag=True)
                # t = muY - R @ muX
                t3 = tmp.tile([P, 3], dt.float32, tag="t3")
                for i in range(3):
                    nc.vector.tensor_scalar(out=t3[:, i:i + 1], in0=R9[:, 3 * i:3 * i + 1],
                                            scalar1=mu[:, 0:1], scalar2=None, op0=op.mult)
                    for j in range(1, 3):
                        nc.vector.scalar_tensor_tensor(
                            out=t3[:, i:i + 1], in0=R9[:, 3 * i + j:3 * i + j + 1],
                            scalar=mu[:, j:j + 1], in1=t3[:, i:i + 1],
                            op0=op.mult, op1=op.add)
                    nc.vector.scalar_tensor_tensor(out=t3[:, i:i + 1], in0=t3[:, i:i + 1],
                                                   scalar=-1.0, in1=mu[:, 3 + i:4 + i],
                                                   op0=op.mult, op1=op.add)
                return R9, t3

            R9, t3 = wproc(mask)

            # dist + score
            d2 = tmp.tile([P, S], dt.float32)
            di = tmp.tile([P, S], dt.float32)
            cc = tmp.tile([P, S], dt.float32)
            nc.vector.memset(d2[:], 0.0)
            for i in range(3):
                nc.vector.tensor_scalar(out=di[:], in0=X[0], scalar1=R9[:, 3 * i:3 * i + 1],
                                        scalar2=None, op0=op.mult)
                for j in range(1, 3):
                    nc.vector.scalar_tensor_tensor(
                        out=di[:], in0=X[j], scalar=R9[:, 3 * i + j:3 * i + j + 1],
                        in1=di[:], op0=op.mult, op1=op.add)
                nc.vector.tensor_scalar(out=di[:], in0=di[:], scalar1=t3[:, i:i + 1],
                                        scalar2=None, op0=op.add)
                nc.vector.tensor_tensor(out=di[:], in0=di[:], in1=Y[i], op=op.subtract)
                nc.vector.tensor_tensor(out=cc[:], in0=di[:], in1=di[:], op=op.mult)
                nc.vector.tensor_tensor(out=d2[:], in0=d2[:], in1=cc[:], op=op.add)
            dd = tmp.tile([P, S], dt.float32)
            nc.scalar.activation(dd[:], d2[:], AF.Sqrt, bias=b0[:, 0:1], scale=1.0)
            score = tmp.tile([P, 1], dt.float32)
            nc.scalar.activation(junk[:], dd[:], AF.Sigmoid, bias=b5[:, 0:1],
                                 scale=-float(BETA), accum_out=score[:])

            # pose loss
            trv = tmp.tile([P, 1], dt.float32)
            nc.vector.scalar_tensor_tensor(out=junk[:, 0:9], in0=R9[:], scalar=1.0,
                                           in1=rgt[:, 0:9], op0=op.mult, op1=op.mult,
                                           accum_out=trv[:])
            cang = tmp.tile([P, 1], dt.float32)
            nc.vector.tensor_scalar(out=cang[:], in0=trv[:], scalar1=-1.0, scalar2=0.5,
                                    op0=op.add, op1=op.mult)
            nc.vector.tensor_scalar(out=cang[:], in0=cang[:], scalar1=0.999999,
                                    scalar2=-0.999999, op0=op.min, op1=op.max)
            s2t = tmp.tile([P, 1], dt.float32)
            nc.vector.scalar_tensor_tensor(out=s2t[:], in0=cang[:], scalar=-1.0,
                                           in1=cang[:], op0=op.mult, op1=op.mult)
            nc.vector.tensor_scalar(out=s2t[:], in0=s2t[:], scalar1=1.0, scalar2=None,
                                    op0=op.add)
            nc.scalar.activation(s2t[:], s2t[:], AF.Sqrt, bias=b0[:, 0:1], scale=1.0)
            nc.vector.reciprocal(s2t[:], s2t[:])
            nc.vector.tensor_tensor(out=s2t[:], in0=cang[:], in1=s2t[:], op=op.mult)
            ang = tmp.tile([P, 1], dt.float32)
            nc.scalar.activation(ang[:], s2t[:], AF.Arctan, bias=b0[:, 0:1], scale=1.0)
            nc.vector.tensor_scalar(out=ang[:], in0=ang[:], scalar1=-1.0,
                                    scalar2=float(np.pi / 2), op0=op.mult, op1=op.add)
            td = tmp.tile([P, 3], dt.float32)
            nc.vector.tensor_tensor(out=td[:], in0=t3[:], in1=rgt[:, 9:12], op=op.subtract)
            terr2 = tmp.tile([P, 1], dt.float32)
            nc.vector.scalar_tensor_tensor(out=junk[:, 0:3], in0=td[:], scalar=1.0,
                                           in1=td[:], op0=op.mult, op1=op.mult,
                                           accum_out=terr2[:])
            terr = tmp.tile([P, 1], dt.float32)
            nc.scalar.activation(terr[:], terr2[:], AF.Sqrt, bias=b0[:, 0:1], scale=1.0)
            lv = tmp.tile([P, 1], dt.float32)
            nc.scalar.activation(lv[:], ang[:], AF.Tanh, bias=b0[:, 0:1], scale=2.0)
            lt = tmp.tile([P, 1], dt.float32)
            nc.scalar.activation(lt[:], terr[:], AF.Tanh, bias=b0[:, 0:1], scale=2.0)
            nc.vector.tensor_tensor(out=lv[:], in0=lv[:], in1=lt[:], op=op.add)
            nc.vector.tensor_scalar(out=lv[:], in0=lv[:], scalar1=0.25, scalar2=None,
                                    op0=op.mult)   # 0.5*(0.5*ta + 0.5*tt)

            # combine: softmax over 8 hyps + null per row
            from concourse.masks import make_identity
            ident = cst.tile([P, P], dt.float32)
            make_identity(nc, ident[:])
            sl = tmp.tile([P, 2], dt.float32)
            nc.vector.tensor_copy(sl[:, 0:1], score[:])
            nc.vector.tensor_copy(sl[:, 1:2], lv[:])
            slT_ps = ps.tile([2, P], dt.float32, space="PSUM")
            nc.tensor.transpose(slT_ps[:], sl[:], ident[:])
            slT = tmp.tile([2, P], dt.float32)
            nc.scalar.copy(slT[:], slT_ps[:])
            sco = tmp.tile([16, 9], dt.float32)
            lvo = tmp.tile([16, 9], dt.float32)
            nc.vector.memset(sco[:], NULLSCORE)
            nc.vector.memset(lvo[:], MAXNULL)
            # [1,128] -> [16,8] via SBUF->SBUF dma
            nc.sync.dma_start(sco[:, 0:8], slT[0:1, :])
            nc.sync.dma_start(lvo[:, 0:8], slT[1:2, :])
            mx = tmp.tile([16, 1], dt.float32)
            nc.vector.tensor_reduce(out=mx[:], in_=sco[:], axis=mybir.AxisListType.X, op=op.max)
            nb = tmp.tile([16, 1], dt.float32)
            nc.vector.tensor_scalar(out=nb[:], in0=mx[:], scalar1=-0.1, scalar2=None,
                                    op0=op.mult)
            e9 = tmp.tile([16, 9], dt.float32)
            esum = tmp.tile([16, 1], dt.float32)
            nc.scalar.activation(e9[:], sco[:], AF.Exp, bias=nb[:, 0:1], scale=0.1,
                                 accum_out=esum[:])
            num = tmp.tile([16, 1], dt.float32)
            junk9 = tmp.tile([16, 9], dt.float32)
            nc.vector.scalar_tensor_tensor(out=junk9[:], in0=lvo[:], scalar=1.0,
                                           in1=e9[:], op0=op.mult, op1=op.mult,
                                           accum_out=num[:])
            nc.vector.reciprocal(esum[:], esum[:])
            tot16 = tmp.tile([16, 1], dt.float32)
            nc.vector.tensor_tensor(out=tot16[:], in0=num[:], in1=esum[:], op=op.mult)
            nc.sync.dma_start(t16_d[:], tot16[:])
            t4 = tmp.tile([BPC, ITM], dt.float32)
            nc.sync.dma_start(t4[:], t16_d.rearrange("(b i) o -> b (i o)", b=BPC))
            red = tmp.tile([BPC, 1], dt.float32)
            nc.vector.tensor_reduce(out=red[:], in_=t4[:], axis=mybir.AxisListType.X, op=op.add)
            nc.vector.tensor_scalar(out=red[:], in0=red[:], scalar1=float(1.0 / ITM),
                                    scalar2=None, op0=op.mult)
            nc.sync.dma_start(out_d[:], red[:])

    nc.finalize()
    _NC_CACHE["nc"] = nc
    return nc


def _host_precompute(matches):
    logm = np.log(matches.reshape(B, NK * NK) + np.float32(1e-12)).astype(np.float32)
    import jax
    import jax.numpy as jnp
    cpu = jax.devices("cpu")[0]

    def gumbel(k, shape):
        u = jax.random.uniform(k, shape, minval=1e-6, maxval=1.0 - 1e-6)
        return np.asarray(-jnp.log(-jnp.log(u)), np.float32)

    v_all = np.empty((ITM, B, NK * NK), np.float32)
    gkr = np.empty((ITM, ITR, B, S), np.float32)
    with jax.default_device(cpu):
        key = jax.random.key(42)
        for it in range(ITM):
            key, km = jax.random.split(key)
            v_all[it] = logm + gumbel(km, (B, NK * NK))
            for k in range(ITR):
                key, kr = jax.random.split(key)
                gkr[it, k] = gumbel(kr, (B, S))
    return logm, v_all, gkr


def _tables(kps, dep, Kinv):
    x, y = kps[:, 0, :], kps[:, 1, :]
    ddep = dep[:, 0, :]
    tab = np.zeros((B, NK, 4), np.float32)
    for i in range(3):
        r = (Kinv[:, i, 0, None] * x + Kinv[:, i, 1, None] * y
             + Kinv[:, i, 2, None]).astype(np.float32)
        tab[:, :, i] = ddep * r
    return tab


def kernel(matches, kps0, depth0, kps1, depth1, K0, K1, Kori_color0, T_0to1):
    from concourse.bass_utils import run_bass_kernel_spmd
    matches = np.asarray(matches, np.float32)
    logm, v_all, gkr = _host_precompute(matches)
    Kinv0 = np.linalg.inv(np.asarray(K0, np.float64)).astype(np.float32)
    Kinv1 = np.linalg.inv(np.asarray(K1, np.float64)).astype(np.float32)
    tab0 = _tables(np.asarray(kps0, np.float32), np.asarray(depth0, np.float32), Kinv0)
    tab1 = _tables(np.asarray(kps1, np.float32), np.asarray(depth1, np.float32), Kinv1)
    T = np.asarray(T_0to1, np.float32)
    Rgt = T[:, :3, :3].reshape(B, 9)
    tgt = T[:, :3, 3]

    in_maps = []
    for c in range(NCORES):
        bs = [4 * c + bc for bc in range(BPC)]
        vrows = np.empty((ROWS, P, FREE), np.float32)
        gk = np.empty((P, S), np.float32)
        rgt = np.empty((P, 12), np.float32)
        for bc, b in enumerate(bs):
            for it in range(ITM):
                r = bc * ITM + it
                vrows[r] = v_all[it, b].reshape(P, FREE)
                for k in range(ITR):
                    q = r * 8 + k
                    gk[q] = gkr[it, k, b]
                    rgt[q, 0:9] = Rgt[b]
                    rgt[q, 9:12] = tgt[b]
        in_maps.append(dict(
            vrows=vrows,
            logm4=logm[bs].reshape(BPC * NK * NK, 1),
            tab0=tab0[bs].reshape(BPC * NK, 4), tab1=tab1[bs].reshape(BPC * NK, 4),
            gk=gk, rgt=rgt,
        ))
    nc = _build_nc()
    trace = bool(os.environ.get("KERNEL_TRACE"))
    res = run_bass_kernel_spmd(nc, in_maps, core_ids=list(range(NCORES)), trace=trace)
    _NC_CACHE["exec_time_ns"] = res.exec_time_ns
    out = np.concatenate([res.results[c]["out"] for c in range(NCORES)], 0)
    return out.astype(np.float32)


# revision 8
# speedup vs baseline: 1.5153x; 1.5153x over previous
"""Trainium2 Bass kernel for nn_MetricPoseLoss: Gumbel top-k match sampling +
RANSAC/Procrustes hypothesis scoring, data-parallel over 8 NeuronCores.

Host side: replicates the reference's Gumbel noise (jax threefry, CPU backend)
and logm = log(matches+1e-12); streams v = logm + gumbel to the device.
Device side (per core, 4 batch elems x 4 sampling iterations = 16 rows):
stream v row tiles, per-partition gumbel-top-4 selection (512 samples/row) via
vector max8/max_index, indirect-DMA gathers of backprojected keypoint pairs
and log-weights, then 8 RANSAC hypotheses per row: gumbel-top-5 minimal sets,
Horn-quaternion weighted Procrustes, inlier scoring, pose loss, and
softmax-with-null combine. Output [32,1] f32.
"""
import os
import numpy as np

B, NK = 32, 1024
S = 512
ITM, ITR = 4, 8
C5 = 5
TH3D = 0.15
BETA = 5.0 / TH3D
TEMP = 10.0
THOUT = 0.35
MAXNULL = 0.5
SCM = 0.5
P = 128
FREE = NK * NK // P  # 8192
NCORES = 8
BPC = B // NCORES    # 4 batches per core
ROWS = BPC * ITM     # 16 rows per core
NULLSCORE = float(np.float32(THOUT) * np.float32(S))

_NC_CACHE = {}


def _build_nc():
    if "nc" in _NC_CACHE:
        return _NC_CACHE["nc"]
    import concourse.bacc as bacc
    import concourse.mybir as mybir
    import concourse.tile as tile
    from concourse.bass import IndirectOffsetOnAxis, AP as BAP

    dt = mybir.dt
    op = mybir.AluOpType
    AF = mybir.ActivationFunctionType

    nc = bacc.Bacc("TRN2", target_bir_lowering=False, debug=False,
                   num_devices=NCORES)
    vrows_d = nc.dram_tensor("vrows", [ROWS, P, FREE], dt.float32, kind="ExternalInput")
    logm_d = nc.dram_tensor("logm4", [BPC * NK * NK, 1], dt.float32, kind="ExternalInput")
    tab0_d = nc.dram_tensor("tab0", [BPC * NK, 4], dt.float32, kind="ExternalInput")
    tab1_d = nc.dram_tensor("tab1", [BPC * NK, 4], dt.float32, kind="ExternalInput")
    gk_d = nc.dram_tensor("gk", [P, S], dt.float32, kind="ExternalInput")
    rgt_d = nc.dram_tensor("rgt", [P, 12], dt.float32, kind="ExternalInput")
    out_d = nc.dram_tensor("out", [BPC, 1], dt.float32, kind="ExternalOutput")
    xrow_d = nc.dram_tensor("xrow", [ROWS, S, 4], dt.float32, kind="Internal")
    yrow_d = nc.dram_tensor("yrow", [ROWS, S, 4], dt.float32, kind="Internal")
    lrow_d = nc.dram_tensor("lrow", [ROWS, S], dt.float32, kind="Internal")
    t16_d = nc.dram_tensor("t16", [ROWS, 1], dt.float32, kind="Internal")

    with tile.TileContext(nc) as tc:
        with (
            tc.tile_pool(name="vpool", bufs=2) as vpool,
            tc.tile_pool(name="sel", bufs=2) as sel,
            tc.tile_pool(name="cst", bufs=1) as cst,
            tc.tile_pool(name="hyp", bufs=1) as hyp,
            tc.tile_pool(name="tmp", bufs=2) as tmp,
            tc.tile_pool(name="ps", bufs=2, space="PSUM") as ps,
        ):
            # constants
            pbase = cst.tile([P, 1], dt.int32)
            nc.gpsimd.iota(pbase[:], [[0, 1]], base=0, channel_multiplier=FREE)
            pbasef = cst.tile([P, 1], dt.float32)
            nc.vector.tensor_copy(pbasef[:], pbase[:])
            ones1 = cst.tile([P, 1], dt.float32)
            nc.vector.memset(ones1[:], 1.0)
            b5 = cst.tile([P, 1], dt.float32)
            nc.vector.memset(b5[:], float(np.float32(BETA) * np.float32(TH3D)))
            b0 = cst.tile([P, 1], dt.float32)
            nc.vector.memset(b0[:], 0.0)
            b0s = cst.tile([16, 1], dt.float32)
            nc.vector.memset(b0s[:], 0.0)

            # ---------- per-row selection + gathers ----------
            for r in range(ROWS):
                bc = r // ITM
                vt = vpool.tile([P, FREE], dt.float32, tag="vt")
                nc.sync.dma_start(vt[:], vrows_d[r])
                m8 = sel.tile([P, 8], dt.float32, tag="m8")
                nc.vector.max(m8[:], vt[:])
                j8 = sel.tile([P, 8], dt.uint32, tag="j8")
                nc.vector.max_index(j8[:], m8[:], vt[:])
                jf = sel.tile([P, 4], dt.float32, tag="jf")
                nc.vector.tensor_copy(jf[:], j8[:, 0:4])
                gidxf = sel.tile([P, 4], dt.float32, tag="gidxf")
                nc.vector.tensor_scalar(out=gidxf[:], in0=jf[:], scalar1=pbasef[:, 0:1],
                                        scalar2=None, op0=op.add)
                gidxi = sel.tile([P, 4], dt.int32, tag="gidxi")
                nc.vector.tensor_copy(gidxi[:], gidxf[:])
                # i0 = floor(gidx/1024) via round-nearest cast of x/1024 - 0.49951171875
                t1 = sel.tile([P, 4], dt.float32, tag="t1")
                nc.vector.tensor_scalar(out=t1[:], in0=gidxf[:], scalar1=float(1.0 / 1024.0),
                                        scalar2=-0.49951171875, op0=op.mult, op1=op.add)
                i0i = sel.tile([P, 4], dt.int32, tag="i0i")
                nc.vector.tensor_copy(i0i[:], t1[:])
                i0f = sel.tile([P, 4], dt.float32, tag="i0f")
                nc.vector.tensor_copy(i0f[:], i0i[:])
                i1f = sel.tile([P, 4], dt.float32, tag="i1f")
                nc.vector.scalar_tensor_tensor(out=i1f[:], in0=i0f[:], scalar=-1024.0,
                                               in1=gidxf[:], op0=op.mult, op1=op.add)
                i1i = sel.tile([P, 4], dt.int32, tag="i1i")
                nc.vector.tensor_copy(i1i[:], i1f[:])

                lw4 = sel.tile([P, 4, 1], dt.float32, tag="lw4")
                xg = sel.tile([P, 4, 4], dt.float32, tag="xg")
                yg = sel.tile([P, 4, 4], dt.float32, tag="yg")
                for s in range(4):
                    nc.gpsimd.indirect_dma_start(
                        out=lw4[:, s, :], out_offset=None,
                        in_=logm_d[:],
                        in_offset=IndirectOffsetOnAxis(ap=gidxi[:, s:s + 1], axis=0),
                        element_offset=bc * NK * NK,
                        bounds_check=NK * NK - 1, oob_is_err=False)
                    nc.gpsimd.indirect_dma_start(
                        out=xg[:, s, :], out_offset=None,
                        in_=tab0_d[:],
                        in_offset=IndirectOffsetOnAxis(ap=i0i[:, s:s + 1], axis=0),
                        element_offset=bc * NK * 4,
                        bounds_check=NK - 1, oob_is_err=False)
                    nc.gpsimd.indirect_dma_start(
                        out=yg[:, s, :], out_offset=None,
                        in_=tab1_d[:],
                        in_offset=IndirectOffsetOnAxis(ap=i1i[:, s:s + 1], axis=0),
                        element_offset=bc * NK * 4,
                        bounds_check=NK - 1, oob_is_err=False)
                nc.scalar.dma_start(xrow_d[r], xg[:])
                nc.scalar.dma_start(yrow_d[r], yg[:])
                nc.scalar.dma_start(lrow_d[r], lw4[:, :, 0])

            # ---------- hypothesis phase ----------
            xh = hyp.tile([P, S, 4], dt.float32)
            yh = hyp.tile([P, S, 4], dt.float32)
            lwh = hyp.tile([P, S], dt.float32)
            def rep8(apx):
                flat = apx.rearrange("s f -> (s f)") if len(apx.shape) == 2 else apx
                return BAP(flat.tensor, flat.offset, [[0, 8]] + list(flat.ap))
            for r in range(ROWS):
                nc.scalar.dma_start(xh[8 * r:8 * r + 8, :, :], rep8(xrow_d[r]))
                nc.scalar.dma_start(yh[8 * r:8 * r + 8, :, :], rep8(yrow_d[r]))
                nc.sync.dma_start(lwh[8 * r:8 * r + 8, :], rep8(lrow_d[r]))
            gk = hyp.tile([P, S], dt.float32)
            nc.sync.dma_start(gk[:], gk_d[:])
            rgt = hyp.tile([P, 12], dt.float32)
            nc.sync.dma_start(rgt[:], rgt_d[:])

            v5 = tmp.tile([P, S], dt.float32)
            nc.vector.tensor_tensor(out=v5[:], in0=lwh[:], in1=gk[:], op=op.add)
            m8b = tmp.tile([P, 8], dt.float32)
            nc.vector.max(m8b[:], v5[:])
            mask = tmp.tile([P, S], dt.float32)
            nc.vector.tensor_scalar(out=mask[:], in0=v5[:], scalar1=m8b[:, 4:5],
                                    scalar2=None, op0=op.is_ge)

            junk = tmp.tile([P, S], dt.float32)
            X = [xh[:, :, i] for i in range(3)]
            Y = [yh[:, :, i] for i in range(3)]

            def wproc(w):
                """weighted procrustes with weights w [P,S]; returns (R9, t3)."""
                wsum = tmp.tile([P, 1], dt.float32, tag="wsum")
                nc.vector.tensor_scalar(out=junk[:], in0=w[:], scalar1=1.0,
                                        scalar2=0.0, op0=op.mult, op1=op.add,
                                        accum_out=wsum[:])
                winv = tmp.tile([P, 1], dt.float32, tag="winv")
                nc.vector.reciprocal(winv[:], wsum[:])
                mu = tmp.tile([P, 6], dt.float32, tag="mu")
                for i in range(3):
                    nc.vector.scalar_tensor_tensor(out=junk[:], in0=X[i], scalar=1.0,
                                                   in1=w[:], op0=op.mult, op1=op.mult,
                                                   accum_out=mu[:, i:i + 1])
                    nc.vector.scalar_tensor_tensor(out=junk[:], in0=Y[i], scalar=1.0,
                                                   in1=w[:], op0=op.mult, op1=op.mult,
                                                   accum_out=mu[:, 3 + i:4 + i])
                nc.vector.tensor_scalar(out=mu[:], in0=mu[:], scalar1=winv[:, 0:1],
                                        scalar2=None, op0=op.mult)
                xc = tmp.tile([P, 3, S], dt.float32, tag="xc")
                yc = tmp.tile([P, 3, S], dt.float32, tag="yc")
                for i in range(3):
                    nc.vector.tensor_scalar(out=xc[:, i, :], in0=X[i], scalar1=mu[:, i:i + 1],
                                            scalar2=None, op0=op.subtract)
                    nc.vector.tensor_scalar(out=yc[:, i, :], in0=Y[i], scalar1=mu[:, 3 + i:4 + i],
                                            scalar2=None, op0=op.subtract)
                    nc.vector.tensor_tensor(out=xc[:, i, :], in0=xc[:, i, :], in1=w[:], op=op.mult)
                H = tmp.tile([P, 9], dt.float32, tag="H")
                for i in range(3):
                    for j in range(3):
                        nc.vector.scalar_tensor_tensor(
                            out=junk[:], in0=xc[:, i, :], scalar=1.0, in1=yc[:, j, :],
                            op0=op.mult, op1=op.mult, accum_out=H[:, 3 * i + j:3 * i + j + 1])
                nc.vector.tensor_scalar(out=H[:], in0=H[:], scalar1=winv[:, 0:1],
                                        scalar2=None, op0=op.mult)
                # Horn N matrix [P,16]
                N = tmp.tile([P, 16], dt.float32, tag="N")
                h = lambda i, j: H[:, 3 * i + j:3 * i + j + 1]
                def setn(k, expr_build):
                    expr_build(N[:, k:k + 1])
                def add2(dst, a, b, sa=1.0, sb=1.0):
                    nc.vector.scalar_tensor_tensor(out=dst, in0=a, scalar=sa, in1=junk[:, 0:1],
                                                   op0=op.mult, op1=op.bypass) if False else None
                # simple helpers with TT ops
                def lin(dst, a, b, sb):
                    # dst = a + sb*b
                    nc.vector.scalar_tensor_tensor(out=dst, in0=b, scalar=sb, in1=a,
                                                   op0=op.mult, op1=op.add)
                tr2 = tmp.tile([P, 4], dt.float32, tag="tr2")
                lin(tr2[:, 0:1], h(0, 0), h(1, 1), 1.0)
                lin(N[:, 0:1], tr2[:, 0:1], h(2, 2), 1.0)        # S00+S11+S22
                lin(N[:, 1:2], h(1, 2), h(2, 1), -1.0)           # S12-S21
                lin(N[:, 2:3], h(2, 0), h(0, 2), -1.0)           # S20-S02
                lin(N[:, 3:4], h(0, 1), h(1, 0), -1.0)           # S01-S10
                nc.vector.tensor_copy(N[:, 4:5], N[:, 1:2])
                lin(tr2[:, 1:2], h(0, 0), h(1, 1), -1.0)
                lin(N[:, 5:6], tr2[:, 1:2], h(2, 2), -1.0)       # S00-S11-S22
                lin(N[:, 6:7], h(0, 1), h(1, 0), 1.0)            # S01+S10
                lin(N[:, 7:8], h(0, 2), h(2, 0), 1.0)            # S02+S20
                nc.vector.tensor_copy(N[:, 8:9], N[:, 2:3])
                nc.vector.tensor_copy(N[:, 9:10], N[:, 6:7])
                lin(tr2[:, 2:3], h(1, 1), h(0, 0), -1.0)
                lin(N[:, 10:11], tr2[:, 2:3], h(2, 2), -1.0)     # -S00+S11-S22
                lin(N[:, 11:12], h(1, 2), h(2, 1), 1.0)          # S12+S21
                nc.vector.tensor_copy(N[:, 12:13], N[:, 3:4])
                nc.vector.tensor_copy(N[:, 13:14], N[:, 7:8])
                nc.vector.tensor_copy(N[:, 14:15], N[:, 11:12])
                lin(tr2[:, 3:4], h(2, 2), h(0, 0), -1.0)
                lin(N[:, 15:16], tr2[:, 3:4], h(1, 1), -1.0)     # -S00-S11+S22
                # shift: sigma = 2*sum|H|
                habs = tmp.tile([P, 9], dt.float32, tag="habs")
                nc.scalar.activation(habs[:], H[:], AF.Abs, bias=b0[:, 0:1], scale=1.0)
                sig = tmp.tile([P, 1], dt.float32, tag="sig")
                nc.vector.tensor_scalar(out=habs[:], in0=habs[:], scalar1=2.0,
                                        scalar2=0.0, op0=op.mult, op1=op.add,
                                        accum_out=sig[:])
                for k in (0, 5, 10, 15):
                    nc.vector.tensor_tensor(out=N[:, k:k + 1], in0=N[:, k:k + 1],
                                            in1=sig[:], op=op.add)
                q = tmp.tile([P, 4], dt.float32, tag="q")
                nc.vector.memset(q[:], 0.5)
                qn = tmp.tile([P, 4], dt.float32, tag="qn")
                ss = tmp.tile([P, 1], dt.float32, tag="ss")
                for _ in range(12):
                    for i in range(4):
                        nc.vector.tensor_scalar(out=qn[:, i:i + 1], in0=N[:, 4 * i:4 * i + 1],
                                                scalar1=q[:, 0:1], scalar2=None, op0=op.mult)
                        for j in range(1, 4):
                            nc.vector.scalar_tensor_tensor(
                                out=qn[:, i:i + 1], in0=N[:, 4 * i + j:4 * i + j + 1],
                                scalar=q[:, j:j + 1], in1=qn[:, i:i + 1],
                                op0=op.mult, op1=op.add)
                    nc.vector.scalar_tensor_tensor(out=junk[:, 0:4], in0=qn[:], scalar=1.0,
                                                   in1=qn[:], op0=op.mult, op1=op.mult,
                                                   accum_out=ss[:])
                    nc.vector.reciprocal(ss[:], ss[:])
                    nc.scalar.activation(ss[:], ss[:], AF.Sqrt, bias=b0[:, 0:1], scale=1.0)
                    nc.vector.tensor_scalar(out=q[:], in0=qn[:], scalar1=ss[:, 0:1],
                                            scalar2=None, op0=op.mult)
                # R from q
                pr = tmp.tile([P, 10], dt.float32, tag="pr")
                pairs = [(0, 0), (1, 1), (2, 2), (3, 3), (1, 2), (1, 3), (2, 3),
                         (0, 1), (0, 2), (0, 3)]
                for k, (a, bq) in enumerate(pairs):
                    nc.vector.tensor_scalar(out=pr[:, k:k + 1], in0=q[:, a:a + 1],
                                            scalar1=q[:, bq:bq + 1], scalar2=None, op0=op.mult)
                R9 = tmp.tile([P, 9], dt.float32, tag="R9")
                ww, xx, yy, zz = 0, 1, 2, 3
                xy, xz, yz = 4, 5, 6
                wx, wy, wz = 7, 8, 9
                def rset(k, p1, p2, s2, diag=False):
                    if diag:
                        # 1 - 2*(p1+p2)
                        nc.vector.tensor_tensor(out=R9[:, k:k + 1], in0=pr[:, p1:p1 + 1],
                                                in1=pr[:, p2:p2 + 1], op=op.add)
                        nc.vector.tensor_scalar(out=R9[:, k:k + 1], in0=R9[:, k:k + 1],
                                                scalar1=-2.0, scalar2=1.0,
                                                op0=op.mult, op1=op.add)
                    else:
                        # 2*(p1 + s2*p2)
                        nc.vector.scalar_tensor_tensor(out=R9[:, k:k + 1],
                                                       in0=pr[:, p2:p2 + 1], scalar=s2,
                                                       in1=pr[:, p1:p1 + 1],
                                                       op0=op.mult, op1=op.add)
                        nc.vector.tensor_scalar(out=R9[:, k:k + 1], in0=R9[:, k:k + 1],
                                                scalar1=2.0, scalar2=None, op0=op.mult)
                rset(0, yy, zz, 0, diag=True)
                rset(1, xy, wz, -1.0)
                rset(2, xz, wy, 1.0)
                rset(3, xy, wz, 1.0)
                rset(4, xx, zz, 0, diag=True)
                rset(5, yz, wx, -1.0)
                rset(6, xz, wy, -1.0)
                rset(7, yz, wx, 1.0)
                rset(8, xx, yy, 0, diag=True)
                # t = muY - R @ muX
                t3 = tmp.tile([P, 3], dt.float32, tag="t3")
                for i in range(3):
                    nc.vector.tensor_scalar(out=t3[:, i:i + 1], in0=R9[:, 3 * i:3 * i + 1],
                                            scalar1=mu[:, 0:1], scalar2=None, op0=op.mult)
                    for j in range(1, 3):
                        nc.vector.scalar_tensor_tensor(
                            out=t3[:, i:i + 1], in0=R9[:, 3 * i + j:3 * i + j + 1],
                            scalar=mu[:, j:j + 1], in1=t3[:, i:i + 1],
                            op0=op.mult, op1=op.add)
                    nc.vector.scalar_tensor_tensor(out=t3[:, i:i + 1], in0=t3[:, i:i + 1],
                                                   scalar=-1.0, in1=mu[:, 3 + i:4 + i],
                                                   op0=op.mult, op1=op.add)
                return R9, t3

            R9, t3 = wproc(mask)

            # dist + score
            d2 = tmp.tile([P, S], dt.float32)
            di = tmp.tile([P, S], dt.float32)
            cc = tmp.tile([P, S], dt.float32)
            nc.vector.memset(d2[:], 0.0)
            for i in range(3):
                nc.vector.tensor_scalar(out=di[:], in0=X[0], scalar1=R9[:, 3 * i:3 * i + 1],
                                        scalar2=None, op0=op.mult)
                for j in range(1, 3):
                    nc.vector.scalar_tensor_tensor(
                        out=di[:], in0=X[j], scalar=R9[:, 3 * i + j:3 * i + j + 1],
                        in1=di[:], op0=op.mult, op1=op.add)
                nc.vector.tensor_scalar(out=di[:], in0=di[:], scalar1=t3[:, i:i + 1],
                                        scalar2=None, op0=op.add)
                nc.vector.tensor_tensor(out=di[:], in0=di[:], in1=Y[i], op=op.subtract)
                nc.vector.tensor_tensor(out=cc[:], in0=di[:], in1=di[:], op=op.mult)
                nc.vector.tensor_tensor(out=d2[:], in0=d2[:], in1=cc[:], op=op.add)
            dd = tmp.tile([P, S], dt.float32)
            nc.scalar.activation(dd[:], d2[:], AF.Sqrt, bias=b0[:, 0:1], scale=1.0)
            score = tmp.tile([P, 1], dt.float32)
            nc.scalar.activation(junk[:], dd[:], AF.Sigmoid, bias=b5[:, 0:1],
                                 scale=-float(BETA), accum_out=score[:])

            # pose loss
            trv = tmp.tile([P, 1], dt.float32)
            nc.vector.scalar_tensor_tensor(out=junk[:, 0:9], in0=R9[:], scalar=1.0,
                                           in1=rgt[:, 0:9], op0=op.mult, op1=op.mult,
                                           accum_out=trv[:])
            cang = tmp.tile([P, 1], dt.float32)
            nc.vector.tensor_scalar(out=cang[:], in0=trv[:], scalar1=-1.0, scalar2=0.5,
                                    op0=op.add, op1=op.mult)
            nc.vector.tensor_scalar(out=cang[:], in0=cang[:], scalar1=0.999999,
                                    scalar2=-0.999999, op0=op.min, op1=op.max)
            s2t = tmp.tile([P, 1], dt.float32)
            nc.vector.scalar_tensor_tensor(out=s2t[:], in0=cang[:], scalar=-1.0,
                                           in1=cang[:], op0=op.mult, op1=op.mult)
            nc.vector.tensor_scalar(out=s2t[:], in0=s2t[:], scalar1=1.0, scalar2=None,
                                    op0=op.add)
            nc.scalar.activation(s2t[:], s2t[:], AF.Sqrt, bias=b0[:, 0:1], scale=1.0)
            nc.vector.reciprocal(s2t[:], s2t[:])
            nc.vector.tensor_tensor(out=s2t[:], in0=cang[:], in1=s2t[:], op=op.mult)
            ang = tmp.tile([P, 1], dt.float32)
            nc.scalar.activation(ang[:], s2t[:], AF.Arctan, bias=b0[:, 0:1], scale=1.0)
            nc.vector.tensor_scalar(out=ang[:], in0=ang[:], scalar1=-1.0,
                                    scalar2=float(np.pi / 2), op0=op.mult, op1=op.add)
            td = tmp.tile([P, 3], dt.float32)
            nc.vector.tensor_tensor(out=td[:], in0=t3[:], in1=rgt[:, 9:12], op=op.subtract)
            terr2 = tmp.tile([P, 1], dt.float32)
            nc.vector.scalar_tensor_tensor(out=junk[:, 0:3], in0=td[:], scalar=1.0,
                                           in1=td[:], op0=op.mult, op1=op.mult,
                                           accum_out=terr2[:])
            terr = tmp.tile([P, 1], dt.float32)
            nc.scalar.activation(terr[:], terr2[:], AF.Sqrt, bias=b0[:, 0:1], scale=1.0)
            lv = tmp.tile([P, 1], dt.float32)
            nc.scalar.activation(lv[:], ang[:], AF.Tanh, bias=b0[:, 0:1], scale=2.0)
            lt = tmp.tile([P, 1], dt.float32)
            nc.scalar.activation(lt[:], terr[:], AF.Tanh, bias=b0[:, 0:1], scale=2.0)
            nc.vector.tensor_tensor(out=lv[:], in0=lv[:], in1=lt[:], op=op.add)
            nc.vector.tensor_scalar(out=lv[:], in0=lv[:], scalar1=0.25, scalar2=None,
                                    op0=op.mult)   # 0.5*(0.5*ta + 0.5*tt)

            # combine: softmax over 8 hyps + null per row
            from concourse.masks import make_identity
            ident = cst.tile([P, P], dt.float32)
            make_identity(nc, ident[:])
            sl = tmp.tile([P, 2], dt.float32)
            nc.vector.tensor_copy(sl[:, 0:1], score[:])
            nc.vector.tensor_copy(sl[:, 1:2], lv[:])
            slT_ps = ps.tile([2, P], dt.float32, space="PSUM")
            nc.tensor.transpose(slT_ps[:], sl[:], ident[:])
            slT = tmp.tile([2, P], dt.float32)
            nc.scalar.copy(slT[:], slT_ps[:])
            sco = tmp.tile([16, 9], dt.float32)
            lvo = tmp.tile([16, 9], dt.float32)
            nc.vector.memset(sco[:], NULLSCORE)
            nc.vector.memset(lvo[:], MAXNULL)
            # [1,128] -> [16,8] via SBUF->SBUF dma
            nc.sync.dma_start(sco[:, 0:8], slT[0:1, :])
            nc.sync.dma_start(lvo[:, 0:8], slT[1:2, :])
            mx = tmp.tile([16, 1], dt.float32)
            nc.vector.tensor_reduce(out=mx[:], in_=sco[:], axis=mybir.AxisListType.X, op=op.max)
            nb = tmp.tile([16, 1], dt.float32)
            nc.vector.tensor_scalar(out=nb[:], in0=mx[:], scalar1=-0.1, scalar2=None,
                                    op0=op.mult)
            e9 = tmp.tile([16, 9], dt.float32)
            esum = tmp.tile([16, 1], dt.float32)
            nc.scalar.activation(e9[:], sco[:], AF.Exp, bias=nb[:, 0:1], scale=0.1,
                                 accum_out=esum[:])
            num = tmp.tile([16, 1], dt.float32)
            junk9 = tmp.tile([16, 9], dt.float32)
            nc.vector.scalar_tensor_tensor(out=junk9[:], in0=lvo[:], scalar=1.0,
                                           in1=e9[:], op0=op.mult, op1=op.mult,
                                           accum_out=num[:])
            nc.vector.reciprocal(esum[:], esum[:])
            tot16 = tmp.tile([16, 1], dt.float32)
            nc.vector.tensor_tensor(out=tot16[:], in0=num[:], in1=esum[:], op=op.mult)
            nc.sync.dma_start(t16_d[:], tot16[:])
            t4 = tmp.tile([BPC, ITM], dt.float32)
            nc.sync.dma_start(t4[:], t16_d.rearrange("(b i) o -> b (i o)", b=BPC))
            red = tmp.tile([BPC, 1], dt.float32)
            nc.vector.tensor_reduce(out=red[:], in_=t4[:], axis=mybir.AxisListType.X, op=op.add)
            nc.vector.tensor_scalar(out=red[:], in0=red[:], scalar1=float(1.0 / ITM),
                                    scalar2=None, op0=op.mult)
            nc.sync.dma_start(out_d[:], red[:])

    nc.finalize()
    _NC_CACHE["nc"] = nc
    return nc


def _host_precompute(matches):
    logm = np.log(matches.reshape(B, NK * NK) + np.float32(1e-12)).astype(np.float32)
    import jax
    import jax.numpy as jnp
    cpu = jax.devices("cpu")[0]

    def gumbel(k, shape):
        u = jax.random.uniform(k, shape, minval=1e-6, maxval=1.0 - 1e-6)
        return np.asarray(-jnp.log(-jnp.log(u)), np.float32)

    v_all = np.empty((ITM, B, NK * NK), np.float32)
    gkr = np.empty((ITM, ITR, B, S), np.float32)
    with jax.default_device(cpu):
        key = jax.random.key(42)
        for it in range(ITM):
            key, km = jax.random.split(key)
            v_all[it] = logm + gumbel(km, (B, NK * NK))
            for k in range(ITR):
                key, kr = jax.random.split(key)
                gkr[it, k] = gumbel(kr, (B, S))
    return logm, v_all, gkr


def _tables(kps, dep, Kinv):
    x, y = kps[:, 0, :], kps[:, 1, :]
    ddep = dep[:, 0, :]
    tab = np.zeros((B, NK, 4), np.float32)
    for i in range(3):
        r = (Kinv[:, i, 0, None] * x + Kinv[:, i, 1, None] * y
             + Kinv[:, i, 2, None]).astype(np.float32)
        tab[:, :, i] = ddep * r
    return tab


def kernel(matches, kps0, depth0, kps1, depth1, K0, K1, Kori_color0, T_0to1):
    from concourse.bass_utils import run_bass_kernel_spmd
    matches = np.asarray(matches, np.float32)
    logm, v_all, gkr = _host_precompute(matches)
    Kinv0 = np.linalg.inv(np.asarray(K0, np.float64)).astype(np.float32)
    Kinv1 = np.linalg.inv(np.asarray(K1, np.float64)).astype(np.float32)
    tab0 = _tables(np.asarray(kps0, np.float32), np.asarray(depth0, np.float32), Kinv0)
    tab1 = _tables(np.asarray(kps1, np.float32), np.asarray(depth1, np.float32), Kinv1)
    T = np.asarray(T_0to1, np.float32)
    Rgt = T[:, :3, :3].reshape(B, 9)
    tgt = T[:, :3, 3]

    in_maps = []
    for c in range(NCORES):
        bs = [4 * c + bc for bc in range(BPC)]
        vrows = np.empty((ROWS, P, FREE), np.float32)
        gk = np.empty((P, S), np.float32)
        rgt = np.empty((P, 12), np.float32)
        for bc, b in enumerate(bs):
            for it in range(ITM):
                r = bc * ITM + it
                vrows[r] = v_all[it, b].reshape(P, FREE)
                for k in range(ITR):
                    q = r * 8 + k
                    gk[q] = gkr[it, k, b]
                    rgt[q, 0:9] = Rgt[b]
                    rgt[q, 9:12] = tgt[b]
        in_maps.append(dict(
            vrows=vrows,
            logm4=logm[bs].reshape(BPC * NK * NK, 1),
            tab0=tab0[bs].reshape(BPC * NK, 4), tab1=tab1[bs].reshape(BPC * NK, 4),
            gk=gk, rgt=rgt,
        ))
    nc = _build_nc()
    trace = bool(os.environ.get("KERNEL_TRACE"))
    res = run_bass_kernel_spmd(nc, in_maps, core_ids=list(range(NCORES)), trace=trace)
    _NC_CACHE["exec_time_ns"] = res.exec_time_ns
    out = np.concatenate([res.results[c]["out"] for c in range(NCORES)], 0)
    return out.astype(np.float32)


# revision 9
# speedup vs baseline: 1.5772x; 1.0409x over previous
"""Trainium2 Bass kernel for nn_MetricPoseLoss: Gumbel top-k match sampling +
RANSAC/Procrustes hypothesis scoring, data-parallel over 8 NeuronCores.

Host side: replicates the reference's Gumbel noise (jax threefry, CPU backend)
and logm = log(matches+1e-12); streams v = logm + gumbel to the device.
Device side (per core, 4 batch elems x 4 sampling iterations = 16 rows):
stream v row tiles, per-partition gumbel-top-4 selection (512 samples/row) via
vector max8/max_index, indirect-DMA gathers of backprojected keypoint pairs
and log-weights, then 8 RANSAC hypotheses per row: gumbel-top-5 minimal sets,
Horn-quaternion weighted Procrustes, inlier scoring, pose loss, and
softmax-with-null combine. Output [32,1] f32.
"""
import os
import numpy as np

B, NK = 32, 1024
S = 512
ITM, ITR = 4, 8
C5 = 5
TH3D = 0.15
BETA = 5.0 / TH3D
TEMP = 10.0
THOUT = 0.35
MAXNULL = 0.5
SCM = 0.5
P = 128
FREE = NK * NK // P  # 8192
NCORES = 8
BPC = B // NCORES    # 4 batches per core
ROWS = BPC * ITM     # 16 rows per core
NULLSCORE = float(np.float32(THOUT) * np.float32(S))

_NC_CACHE = {}


def _build_nc():
    if "nc" in _NC_CACHE:
        return _NC_CACHE["nc"]
    import concourse.bacc as bacc
    import concourse.mybir as mybir
    import concourse.tile as tile
    from concourse.bass import IndirectOffsetOnAxis, AP as BAP

    dt = mybir.dt
    op = mybir.AluOpType
    AF = mybir.ActivationFunctionType

    nc = bacc.Bacc("TRN2", target_bir_lowering=False, debug=False,
                   num_devices=NCORES)
    vrows_d = nc.dram_tensor("vrows", [ROWS, P, FREE], dt.float32, kind="ExternalInput")
    logm_d = nc.dram_tensor("logm4", [BPC * NK * NK, 1], dt.float32, kind="ExternalInput")
    tab0_d = nc.dram_tensor("tab0", [BPC * NK, 4], dt.float32, kind="ExternalInput")
    tab1_d = nc.dram_tensor("tab1", [BPC * NK, 4], dt.float32, kind="ExternalInput")
    gk_d = nc.dram_tensor("gk", [P, S], dt.float32, kind="ExternalInput")
    rgt_d = nc.dram_tensor("rgt", [P, 12], dt.float32, kind="ExternalInput")
    out_d = nc.dram_tensor("out", [BPC, 1], dt.float32, kind="ExternalOutput")
    xrow_d = nc.dram_tensor("xrow", [ROWS, S, 4], dt.float32, kind="Internal")
    yrow_d = nc.dram_tensor("yrow", [ROWS, S, 4], dt.float32, kind="Internal")
    lrow_d = nc.dram_tensor("lrow", [ROWS, S], dt.float32, kind="Internal")
    t16_d = nc.dram_tensor("t16", [ROWS, 1], dt.float32, kind="Internal")

    with tile.TileContext(nc) as tc:
        with (
            tc.tile_pool(name="vpool", bufs=3) as vpool,
            tc.tile_pool(name="sel", bufs=3) as sel,
            tc.tile_pool(name="cst", bufs=1) as cst,
            tc.tile_pool(name="hyp", bufs=1) as hyp,
            tc.tile_pool(name="tmp", bufs=2) as tmp,
            tc.tile_pool(name="ps", bufs=2, space="PSUM") as ps,
        ):
            # constants
            pbase = cst.tile([P, 1], dt.int32)
            nc.gpsimd.iota(pbase[:], [[0, 1]], base=0, channel_multiplier=FREE)
            pbasef = cst.tile([P, 1], dt.float32)
            nc.vector.tensor_copy(pbasef[:], pbase[:])
            ones1 = cst.tile([P, 1], dt.float32)
            nc.vector.memset(ones1[:], 1.0)
            b5 = cst.tile([P, 1], dt.float32)
            nc.vector.memset(b5[:], float(np.float32(BETA) * np.float32(TH3D)))
            b0 = cst.tile([P, 1], dt.float32)
            nc.vector.memset(b0[:], 0.0)
            b0s = cst.tile([16, 1], dt.float32)
            nc.vector.memset(b0s[:], 0.0)

            # ---------- per-row selection + gathers ----------
            for r in range(ROWS):
                bc = r // ITM
                vt = vpool.tile([P, FREE], dt.float32, tag="vt")
                nc.sync.dma_start(vt[:], vrows_d[r])
                m8 = sel.tile([P, 8], dt.float32, tag="m8")
                nc.vector.max(m8[:], vt[:])
                j8 = sel.tile([P, 8], dt.uint32, tag="j8")
                nc.vector.max_index(j8[:], m8[:], vt[:])
                jf = sel.tile([P, 4], dt.float32, tag="jf")
                nc.vector.tensor_copy(jf[:], j8[:, 0:4])
                gidxf = sel.tile([P, 4], dt.float32, tag="gidxf")
                nc.vector.tensor_scalar(out=gidxf[:], in0=jf[:], scalar1=pbasef[:, 0:1],
                                        scalar2=None, op0=op.add)
                gidxi = sel.tile([P, 4], dt.int32, tag="gidxi")
                nc.vector.tensor_copy(gidxi[:], gidxf[:])
                # i0 = floor(gidx/1024) via round-nearest cast of x/1024 - 0.49951171875
                t1 = sel.tile([P, 4], dt.float32, tag="t1")
                nc.vector.tensor_scalar(out=t1[:], in0=gidxf[:], scalar1=float(1.0 / 1024.0),
                                        scalar2=-0.49951171875, op0=op.mult, op1=op.add)
                i0i = sel.tile([P, 4], dt.int32, tag="i0i")
                nc.vector.tensor_copy(i0i[:], t1[:])
                i0f = sel.tile([P, 4], dt.float32, tag="i0f")
                nc.vector.tensor_copy(i0f[:], i0i[:])
                i1f = sel.tile([P, 4], dt.float32, tag="i1f")
                nc.vector.scalar_tensor_tensor(out=i1f[:], in0=i0f[:], scalar=-1024.0,
                                               in1=gidxf[:], op0=op.mult, op1=op.add)
                i1i = sel.tile([P, 4], dt.int32, tag="i1i")
                nc.vector.tensor_copy(i1i[:], i1f[:])

                lw4 = sel.tile([P, 4, 1], dt.float32, tag="lw4")
                xg = sel.tile([P, 4, 4], dt.float32, tag="xg")
                yg = sel.tile([P, 4, 4], dt.float32, tag="yg")
                for s in range(4):
                    nc.gpsimd.indirect_dma_start(
                        out=lw4[:, s, :], out_offset=None,
                        in_=logm_d[:],
                        in_offset=IndirectOffsetOnAxis(ap=gidxi[:, s:s + 1], axis=0),
                        element_offset=bc * NK * NK,
                        bounds_check=NK * NK - 1, oob_is_err=False)
                    nc.gpsimd.indirect_dma_start(
                        out=xg[:, s, :], out_offset=None,
                        in_=tab0_d[:],
                        in_offset=IndirectOffsetOnAxis(ap=i0i[:, s:s + 1], axis=0),
                        element_offset=bc * NK * 4,
                        bounds_check=NK - 1, oob_is_err=False)
                    nc.gpsimd.indirect_dma_start(
                        out=yg[:, s, :], out_offset=None,
                        in_=tab1_d[:],
                        in_offset=IndirectOffsetOnAxis(ap=i1i[:, s:s + 1], axis=0),
                        element_offset=bc * NK * 4,
                        bounds_check=NK - 1, oob_is_err=False)
                nc.scalar.dma_start(xrow_d[r], xg[:])
                nc.scalar.dma_start(yrow_d[r], yg[:])
                nc.scalar.dma_start(lrow_d[r], lw4[:, :, 0])

            # ---------- hypothesis phase ----------
            xh = hyp.tile([P, S, 4], dt.float32)
            yh = hyp.tile([P, S, 4], dt.float32)
            lwh = hyp.tile([P, S], dt.float32)
            def rep8(apx):
                flat = apx.rearrange("s f -> (s f)") if len(apx.shape) == 2 else apx
                return BAP(flat.tensor, flat.offset, [[0, 8]] + list(flat.ap))
            for r in range(ROWS):
                nc.scalar.dma_start(xh[8 * r:8 * r + 8, :, :], rep8(xrow_d[r]))
                nc.scalar.dma_start(yh[8 * r:8 * r + 8, :, :], rep8(yrow_d[r]))
                nc.sync.dma_start(lwh[8 * r:8 * r + 8, :], rep8(lrow_d[r]))
            gk = hyp.tile([P, S], dt.float32)
            nc.sync.dma_start(gk[:], gk_d[:])
            rgt = hyp.tile([P, 12], dt.float32)
            nc.sync.dma_start(rgt[:], rgt_d[:])

            v5 = tmp.tile([P, S], dt.float32)
            nc.vector.tensor_tensor(out=v5[:], in0=lwh[:], in1=gk[:], op=op.add)
            m8b = tmp.tile([P, 8], dt.float32)
            nc.vector.max(m8b[:], v5[:])
            mask = tmp.tile([P, S], dt.float32)
            nc.vector.tensor_scalar(out=mask[:], in0=v5[:], scalar1=m8b[:, 4:5],
                                    scalar2=None, op0=op.is_ge)

            junk = tmp.tile([P, S], dt.float32)
            X = [xh[:, :, i] for i in range(3)]
            Y = [yh[:, :, i] for i in range(3)]

            def wproc(w):
                """weighted procrustes with weights w [P,S]; returns (R9, t3)."""
                wsum = tmp.tile([P, 1], dt.float32, tag="wsum")
                nc.vector.tensor_scalar(out=junk[:], in0=w[:], scalar1=1.0,
                                        scalar2=0.0, op0=op.mult, op1=op.add,
                                        accum_out=wsum[:])
                winv = tmp.tile([P, 1], dt.float32, tag="winv")
                nc.vector.reciprocal(winv[:], wsum[:])
                mu = tmp.tile([P, 6], dt.float32, tag="mu")
                for i in range(3):
                    nc.vector.scalar_tensor_tensor(out=junk[:], in0=X[i], scalar=1.0,
                                                   in1=w[:], op0=op.mult, op1=op.mult,
                                                   accum_out=mu[:, i:i + 1])
                    nc.vector.scalar_tensor_tensor(out=junk[:], in0=Y[i], scalar=1.0,
                                                   in1=w[:], op0=op.mult, op1=op.mult,
                                                   accum_out=mu[:, 3 + i:4 + i])
                nc.vector.tensor_scalar(out=mu[:], in0=mu[:], scalar1=winv[:, 0:1],
                                        scalar2=None, op0=op.mult)
                xc = tmp.tile([P, 3, S], dt.float32, tag="xc")
                yc = tmp.tile([P, 3, S], dt.float32, tag="yc")
                for i in range(3):
                    nc.vector.tensor_scalar(out=xc[:, i, :], in0=X[i], scalar1=mu[:, i:i + 1],
                                            scalar2=None, op0=op.subtract)
                    nc.vector.tensor_scalar(out=yc[:, i, :], in0=Y[i], scalar1=mu[:, 3 + i:4 + i],
                                            scalar2=None, op0=op.subtract)
                    nc.vector.tensor_tensor(out=xc[:, i, :], in0=xc[:, i, :], in1=w[:], op=op.mult)
                H = tmp.tile([P, 9], dt.float32, tag="H")
                for i in range(3):
                    for j in range(3):
                        nc.vector.scalar_tensor_tensor(
                            out=junk[:], in0=xc[:, i, :], scalar=1.0, in1=yc[:, j, :],
                            op0=op.mult, op1=op.mult, accum_out=H[:, 3 * i + j:3 * i + j + 1])
                nc.vector.tensor_scalar(out=H[:], in0=H[:], scalar1=winv[:, 0:1],
                                        scalar2=None, op0=op.mult)
                # Horn N matrix [P,16]
                N = tmp.tile([P, 16], dt.float32, tag="N")
                h = lambda i, j: H[:, 3 * i + j:3 * i + j + 1]
                def setn(k, expr_build):
                    expr_build(N[:, k:k + 1])
                def add2(dst, a, b, sa=1.0, sb=1.0):
                    nc.vector.scalar_tensor_tensor(out=dst, in0=a, scalar=sa, in1=junk[:, 0:1],
                                                   op0=op.mult, op1=op.bypass) if False else None
                # simple helpers with TT ops
                def lin(dst, a, b, sb):
                    # dst = a + sb*b
                    nc.vector.scalar_tensor_tensor(out=dst, in0=b, scalar=sb, in1=a,
                                                   op0=op.mult, op1=op.add)
                tr2 = tmp.tile([P, 4], dt.float32, tag="tr2")
                lin(tr2[:, 0:1], h(0, 0), h(1, 1), 1.0)
                lin(N[:, 0:1], tr2[:, 0:1], h(2, 2), 1.0)        # S00+S11+S22
                lin(N[:, 1:2], h(1, 2), h(2, 1), -1.0)           # S12-S21
                lin(N[:, 2:3], h(2, 0), h(0, 2), -1.0)           # S20-S02
                lin(N[:, 3:4], h(0, 1), h(1, 0), -1.0)           # S01-S10
                nc.vector.tensor_copy(N[:, 4:5], N[:, 1:2])
                lin(tr2[:, 1:2], h(0, 0), h(1, 1), -1.0)
                lin(N[:, 5:6], tr2[:, 1:2], h(2, 2), -1.0)       # S00-S11-S22
                lin(N[:, 6:7], h(0, 1), h(1, 0), 1.0)            # S01+S10
                lin(N[:, 7:8], h(0, 2), h(2, 0), 1.0)            # S02+S20
                nc.vector.tensor_copy(N[:, 8:9], N[:, 2:3])
                nc.vector.tensor_copy(N[:, 9:10], N[:, 6:7])
                lin(tr2[:, 2:3], h(1, 1), h(0, 0), -1.0)
                lin(N[:, 10:11], tr2[:, 2:3], h(2, 2), -1.0)     # -S00+S11-S22
                lin(N[:, 11:12], h(1, 2), h(2, 1), 1.0)          # S12+S21
                nc.vector.tensor_copy(N[:, 12:13], N[:, 3:4])
                nc.vector.tensor_copy(N[:, 13:14], N[:, 7:8])
                nc.vector.tensor_copy(N[:, 14:15], N[:, 11:12])
                lin(tr2[:, 3:4], h(2, 2), h(0, 0), -1.0)
                lin(N[:, 15:16], tr2[:, 3:4], h(1, 1), -1.0)     # -S00-S11+S22
                # shift: sigma = 2*sum|H|
                habs = tmp.tile([P, 9], dt.float32, tag="habs")
                nc.scalar.activation(habs[:], H[:], AF.Abs, bias=b0[:, 0:1], scale=1.0)
                sig = tmp.tile([P, 1], dt.float32, tag="sig")
                nc.vector.tensor_scalar(out=habs[:], in0=habs[:], scalar1=2.0,
                                        scalar2=0.0, op0=op.mult, op1=op.add,
                                        accum_out=sig[:])
                for k in (0, 5, 10, 15):
                    nc.vector.tensor_tensor(out=N[:, k:k + 1], in0=N[:, k:k + 1],
                                            in1=sig[:], op=op.add)
                q = tmp.tile([P, 4], dt.float32, tag="q")
                nc.vector.memset(q[:], 0.5)
                qn = tmp.tile([P, 4], dt.float32, tag="qn")
                ss = tmp.tile([P, 1], dt.float32, tag="ss")
                for _ in range(12):
                    for i in range(4):
                        nc.vector.tensor_scalar(out=qn[:, i:i + 1], in0=N[:, 4 * i:4 * i + 1],
                                                scalar1=q[:, 0:1], scalar2=None, op0=op.mult)
                        for j in range(1, 4):
                            nc.vector.scalar_tensor_tensor(
                                out=qn[:, i:i + 1], in0=N[:, 4 * i + j:4 * i + j + 1],
                                scalar=q[:, j:j + 1], in1=qn[:, i:i + 1],
                                op0=op.mult, op1=op.add)
                    nc.vector.scalar_tensor_tensor(out=junk[:, 0:4], in0=qn[:], scalar=1.0,
                                                   in1=qn[:], op0=op.mult, op1=op.mult,
                                                   accum_out=ss[:])
                    nc.vector.reciprocal(ss[:], ss[:])
                    nc.scalar.activation(ss[:], ss[:], AF.Sqrt, bias=b0[:, 0:1], scale=1.0)
                    nc.vector.tensor_scalar(out=q[:], in0=qn[:], scalar1=ss[:, 0:1],
                                            scalar2=None, op0=op.mult)
                # R from q
                pr = tmp.tile([P, 10], dt.float32, tag="pr")
                pairs = [(0, 0), (1, 1), (2, 2), (3, 3), (1, 2), (1, 3), (2, 3),
                         (0, 1), (0, 2), (0, 3)]
                for k, (a, bq) in enumerate(pairs):
                    nc.vector.tensor_scalar(out=pr[:, k:k + 1], in0=q[:, a:a + 1],
                                            scalar1=q[:, bq:bq + 1], scalar2=None, op0=op.mult)
                R9 = tmp.tile([P, 9], dt.float32, tag="R9")
                ww, xx, yy, zz = 0, 1, 2, 3
                xy, xz, yz = 4, 5, 6
                wx, wy, wz = 7, 8, 9
                def rset(k, p1, p2, s2, diag=False):
                    if diag:
                        # 1 - 2*(p1+p2)
                        nc.vector.tensor_tensor(out=R9[:, k:k + 1], in0=pr[:, p1:p1 + 1],
                                                in1=pr[:, p2:p2 + 1], op=op.add)
                        nc.vector.tensor_scalar(out=R9[:, k:k + 1], in0=R9[:, k:k + 1],
                                                scalar1=-2.0, scalar2=1.0,
                                                op0=op.mult, op1=op.add)
                    else:
                        # 2*(p1 + s2*p2)
                        nc.vector.scalar_tensor_tensor(out=R9[:, k:k + 1],
                                                       in0=pr[:, p2:p2 + 1], scalar=s2,
                                                       in1=pr[:, p1:p1 + 1],
                                                       op0=op.mult, op1=op.add)
                        nc.vector.tensor_scalar(out=R9[:, k:k + 1], in0=R9[:, k:k + 1],
                                                scalar1=2.0, scalar2=None, op0=op.mult)
                rset(0, yy, zz, 0, diag=True)
                rset(1, xy, wz, -1.0)
                rset(2, xz, wy, 1.0)
                rset(3, xy, wz, 1.0)
                rset(4, xx, zz, 0, diag=True)
                rset(5, yz, wx, -1.0)
                rset(6, xz, wy, -1.0)
                rset(7, yz, wx, 1.0)
                rset(8, xx, yy, 0, diag=True)
                # t = muY - R @ muX
                t3 = tmp.tile([P, 3], dt.float32, tag="t3")
                for i in range(3):
                    nc.vector.tensor_scalar(out=t3[:, i:i + 1], in0=R9[:, 3 * i:3 * i + 1],
                                            scalar1=mu[:, 0:1], scalar2=None, op0=op.mult)
                    for j in range(1, 3):
                        nc.vector.scalar_tensor_tensor(
                            out=t3[:, i:i + 1], in0=R9[:, 3 * i + j:3 * i + j + 1],
                            scalar=mu[:, j:j + 1], in1=t3[:, i:i + 1],
                            op0=op.mult, op1=op.add)
                    nc.vector.scalar_tensor_tensor(out=t3[:, i:i + 1], in0=t3[:, i:i + 1],
                                                   scalar=-1.0, in1=mu[:, 3 + i:4 + i],
                                                   op0=op.mult, op1=op.add)
                return R9, t3

            R9, t3 = wproc(mask)

            # dist + score
            d2 = tmp.tile([P, S], dt.float32)
            di = tmp.tile([P, S], dt.float32)
            cc = tmp.tile([P, S], dt.float32)
            nc.vector.memset(d2[:], 0.0)
            for i in range(3):
                nc.vector.tensor_scalar(out=di[:], in0=X[0], scalar1=R9[:, 3 * i:3 * i + 1],
                                        scalar2=None, op0=op.mult)
                for j in range(1, 3):
                    nc.vector.scalar_tensor_tensor(
                        out=di[:], in0=X[j], scalar=R9[:, 3 * i + j:3 * i + j + 1],
                        in1=di[:], op0=op.mult, op1=op.add)
                nc.vector.tensor_scalar(out=di[:], in0=di[:], scalar1=t3[:, i:i + 1],
                                        scalar2=None, op0=op.add)
                nc.vector.tensor_tensor(out=di[:], in0=di[:], in1=Y[i], op=op.subtract)
                nc.vector.tensor_tensor(out=cc[:], in0=di[:], in1=di[:], op=op.mult)
                nc.vector.tensor_tensor(out=d2[:], in0=d2[:], in1=cc[:], op=op.add)
            dd = tmp.tile([P, S], dt.float32)
            nc.scalar.activation(dd[:], d2[:], AF.Sqrt, bias=b0[:, 0:1], scale=1.0)
            score = tmp.tile([P, 1], dt.float32)
            nc.scalar.activation(junk[:], dd[:], AF.Sigmoid, bias=b5[:, 0:1],
                                 scale=-float(BETA), accum_out=score[:])

            # pose loss
            trv = tmp.tile([P, 1], dt.float32)
            nc.vector.scalar_tensor_tensor(out=junk[:, 0:9], in0=R9[:], scalar=1.0,
                                           in1=rgt[:, 0:9], op0=op.mult, op1=op.mult,
                                           accum_out=trv[:])
            cang = tmp.tile([P, 1], dt.float32)
            nc.vector.tensor_scalar(out=cang[:], in0=trv[:], scalar1=-1.0, scalar2=0.5,
                                    op0=op.add, op1=op.mult)
            nc.vector.tensor_scalar(out=cang[:], in0=cang[:], scalar1=0.999999,
                                    scalar2=-0.999999, op0=op.min, op1=op.max)
            s2t = tmp.tile([P, 1], dt.float32)
            nc.vector.scalar_tensor_tensor(out=s2t[:], in0=cang[:], scalar=-1.0,
                                           in1=cang[:], op0=op.mult, op1=op.mult)
            nc.vector.tensor_scalar(out=s2t[:], in0=s2t[:], scalar1=1.0, scalar2=None,
                                    op0=op.add)
            nc.scalar.activation(s2t[:], s2t[:], AF.Sqrt, bias=b0[:, 0:1], scale=1.0)
            nc.vector.reciprocal(s2t[:], s2t[:])
            nc.vector.tensor_tensor(out=s2t[:], in0=cang[:], in1=s2t[:], op=op.mult)
            ang = tmp.tile([P, 1], dt.float32)
            nc.scalar.activation(ang[:], s2t[:], AF.Arctan, bias=b0[:, 0:1], scale=1.0)
            nc.vector.tensor_scalar(out=ang[:], in0=ang[:], scalar1=-1.0,
                                    scalar2=float(np.pi / 2), op0=op.mult, op1=op.add)
            td = tmp.tile([P, 3], dt.float32)
            nc.vector.tensor_tensor(out=td[:], in0=t3[:], in1=rgt[:, 9:12], op=op.subtract)
            terr2 = tmp.tile([P, 1], dt.float32)
            nc.vector.scalar_tensor_tensor(out=junk[:, 0:3], in0=td[:], scalar=1.0,
                                           in1=td[:], op0=op.mult, op1=op.mult,
                                           accum_out=terr2[:])
            terr = tmp.tile([P, 1], dt.float32)
            nc.scalar.activation(terr[:], terr2[:], AF.Sqrt, bias=b0[:, 0:1], scale=1.0)
            lv = tmp.tile([P, 1], dt.float32)
            nc.scalar.activation(lv[:], ang[:], AF.Tanh, bias=b0[:, 0:1], scale=2.0)
            lt = tmp.tile([P, 1], dt.float32)
            nc.scalar.activation(lt[:], terr[:], AF.Tanh, bias=b0[:, 0:1], scale=2.0)
            nc.vector.tensor_tensor(out=lv[:], in0=lv[:], in1=lt[:], op=op.add)
            nc.vector.tensor_scalar(out=lv[:], in0=lv[:], scalar1=0.25, scalar2=None,
                                    op0=op.mult)   # 0.5*(0.5*ta + 0.5*tt)

            # combine: softmax over 8 hyps + null per row
            from concourse.masks import make_identity
            ident = cst.tile([P, P], dt.float32)
            make_identity(nc, ident[:])
            sl = tmp.tile([P, 2], dt.float32)
            nc.vector.tensor_copy(sl[:, 0:1], score[:])
            nc.vector.tensor_copy(sl[:, 1:2], lv[:])
            slT_ps = ps.tile([2, P], dt.float32, space="PSUM")
            nc.tensor.transpose(slT_ps[:], sl[:], ident[:])
            slT = tmp.tile([2, P], dt.float32)
            nc.scalar.copy(slT[:], slT_ps[:])
            sco = tmp.tile([16, 9], dt.float32)
            lvo = tmp.tile([16, 9], dt.float32)
            nc.vector.memset(sco[:], NULLSCORE)
            nc.vector.memset(lvo[:], MAXNULL)
            # [1,128] -> [16,8] via SBUF->SBUF dma
            nc.sync.dma_start(sco[:, 0:8], slT[0:1, :])
            nc.sync.dma_start(lvo[:, 0:8], slT[1:2, :])
            mx = tmp.tile([16, 1], dt.float32)
            nc.vector.tensor_reduce(out=mx[:], in_=sco[:], axis=mybir.AxisListType.X, op=op.max)
            nb = tmp.tile([16, 1], dt.float32)
            nc.vector.tensor_scalar(out=nb[:], in0=mx[:], scalar1=-0.1, scalar2=None,
                                    op0=op.mult)
            e9 = tmp.tile([16, 9], dt.float32)
            esum = tmp.tile([16, 1], dt.float32)
            nc.scalar.activation(e9[:], sco[:], AF.Exp, bias=nb[:, 0:1], scale=0.1,
                                 accum_out=esum[:])
            num = tmp.tile([16, 1], dt.float32)
            junk9 = tmp.tile([16, 9], dt.float32)
            nc.vector.scalar_tensor_tensor(out=junk9[:], in0=lvo[:], scalar=1.0,
                                           in1=e9[:], op0=op.mult, op1=op.mult,
                                           accum_out=num[:])
            nc.vector.reciprocal(esum[:], esum[:])
            tot16 = tmp.tile([16, 1], dt.float32)
            nc.vector.tensor_tensor(out=tot16[:], in0=num[:], in1=esum[:], op=op.mult)
            nc.sync.dma_start(t16_d[:], tot16[:])
            t4 = tmp.tile([BPC, ITM], dt.float32)
            nc.sync.dma_start(t4[:], t16_d.rearrange("(b i) o -> b (i o)", b=BPC))
            red = tmp.tile([BPC, 1], dt.float32)
            nc.vector.tensor_reduce(out=red[:], in_=t4[:], axis=mybir.AxisListType.X, op=op.add)
            nc.vector.tensor_scalar(out=red[:], in0=red[:], scalar1=float(1.0 / ITM),
                                    scalar2=None, op0=op.mult)
            nc.sync.dma_start(out_d[:], red[:])

    nc.finalize()
    _NC_CACHE["nc"] = nc
    return nc


def _host_precompute(matches):
    logm = np.log(matches.reshape(B, NK * NK) + np.float32(1e-12)).astype(np.float32)
    import jax
    import jax.numpy as jnp
    cpu = jax.devices("cpu")[0]

    def gumbel(k, shape):
        u = jax.random.uniform(k, shape, minval=1e-6, maxval=1.0 - 1e-6)
        return np.asarray(-jnp.log(-jnp.log(u)), np.float32)

    v_all = np.empty((ITM, B, NK * NK), np.float32)
    gkr = np.empty((ITM, ITR, B, S), np.float32)
    with jax.default_device(cpu):
        key = jax.random.key(42)
        for it in range(ITM):
            key, km = jax.random.split(key)
            v_all[it] = logm + gumbel(km, (B, NK * NK))
            for k in range(ITR):
                key, kr = jax.random.split(key)
                gkr[it, k] = gumbel(kr, (B, S))
    return logm, v_all, gkr


def _tables(kps, dep, Kinv):
    x, y = kps[:, 0, :], kps[:, 1, :]
    ddep = dep[:, 0, :]
    tab = np.zeros((B, NK, 4), np.float32)
    for i in range(3):
        r = (Kinv[:, i, 0, None] * x + Kinv[:, i, 1, None] * y
             + Kinv[:, i, 2, None]).astype(np.float32)
        tab[:, :, i] = ddep * r
    return tab


def kernel(matches, kps0, depth0, kps1, depth1, K0, K1, Kori_color0, T_0to1):
    from concourse.bass_utils import run_bass_kernel_spmd
    matches = np.asarray(matches, np.float32)
    logm, v_all, gkr = _host_precompute(matches)
    Kinv0 = np.linalg.inv(np.asarray(K0, np.float64)).astype(np.float32)
    Kinv1 = np.linalg.inv(np.asarray(K1, np.float64)).astype(np.float32)
    tab0 = _tables(np.asarray(kps0, np.float32), np.asarray(depth0, np.float32), Kinv0)
    tab1 = _tables(np.asarray(kps1, np.float32), np.asarray(depth1, np.float32), Kinv1)
    T = np.asarray(T_0to1, np.float32)
    Rgt = T[:, :3, :3].reshape(B, 9)
    tgt = T[:, :3, 3]

    in_maps = []
    for c in range(NCORES):
        bs = [4 * c + bc for bc in range(BPC)]
        vrows = np.empty((ROWS, P, FREE), np.float32)
        gk = np.empty((P, S), np.float32)
        rgt = np.empty((P, 12), np.float32)
        for bc, b in enumerate(bs):
            for it in range(ITM):
                r = bc * ITM + it
                vrows[r] = v_all[it, b].reshape(P, FREE)
                for k in range(ITR):
                    q = r * 8 + k
                    gk[q] = gkr[it, k, b]
                    rgt[q, 0:9] = Rgt[b]
                    rgt[q, 9:12] = tgt[b]
        in_maps.append(dict(
            vrows=vrows,
            logm4=logm[bs].reshape(BPC * NK * NK, 1),
            tab0=tab0[bs].reshape(BPC * NK, 4), tab1=tab1[bs].reshape(BPC * NK, 4),
            gk=gk, rgt=rgt,
        ))
    nc = _build_nc()
    trace = bool(os.environ.get("KERNEL_TRACE"))
    res = run_bass_kernel_spmd(nc, in_maps, core_ids=list(range(NCORES)), trace=trace)
    _NC_CACHE["exec_time_ns"] = res.exec_time_ns
    out = np.concatenate([res.results[c]["out"] for c in range(NCORES)], 0)
    return out.astype(np.float32)
